# revision 8
# baseline (speedup 1.0000x reference)
"""MoE (dense routing) Trainium2 kernel.

Math: out = softmax(x@Wg+bg) -weighted sum over experts of
      (gelu(x@W1[e]+b1[e]) @ W2[e] + b2[e]).

Strategy (data-parallel over 8 cores, 2048 tokens each):
  - Host pre-transposes x (xT [D, tokens]) and packs W1 as [D, E*H].
  - Layer 1 runs "transposed": hT[ej, t] = sum_d W1p[d, ej] * xT[d, t]
    via matmuls with W1p chunks stationary and xT chunks moving ->
    hidden lands with ej on partitions, tokens on free dim.
  - b1 is applied as the ACT bias during the gelu (per-partition bias),
    so it is free.
  - Gate: logitsT[e, t] accumulated the same way; exp fused with +bg on
    ACT; weights kept UNNORMALIZED (exp). The softmax denominator is
    applied at the very end as a per-token scale on the output copy.
  - Scaled hidden shT[ej, t] = gelu_out * exp[e(ej), t] (DVE mul with a
    DMA partition-broadcast of the exp row).
  - Layer 2: out[t, o] = sum_ej shT[ej, t(chunk stationary)] @ W2p[ej, o]
    accumulated in PSUM over all ej chunks, seeded with expT @ b2
    (start=True) which realizes the sum_e w_e*b2[e] term.
  - Final: ACT copy PSUM->SBUF with scale = 1/sum_e exp (per-token,
    computed via a K=8 ones matmul into token-partition layout).
No transposes on device at all.
"""

import numpy as np
from contextlib import ExitStack

import orjson

import concourse.bass as bass
import concourse.bass2jax as bass2jax
import concourse.bass_utils as bass_utils
import concourse.tile as tile
from concourse import mybir
from concourse.bass_utils import run_bass_kernel_spmd

# The walrus build in this container rejects any instruction carrying more
# than one sync wait ("Too many sync wait commands", CoreV3GenImpl
# setupSyncWait), but the tile scheduler freely attaches several. Split the
# extras onto standalone single-wait EventSemaphore carriers placed just
# before the instruction (same engine, so program order is preserved).
_orig_compile_bir_kernel = bass_utils.compile_bir_kernel


def _split_multiwait_bir(bir_json):
    bir = orjson.loads(bir_json)
    changed = False
    for fn in bir.get("functions", []):
        for blk in fn.get("blocks", []):
            ins_list = blk.get("instructions")
            if not ins_list:
                continue
            out = []
            for inst in ins_list:
                si = inst.get("sync_info")
                if si:
                    waits = si.get("on_wait") or []
                    if len(waits) > 1:
                        changed = True
                        for k, w in enumerate(waits[:-1]):
                            carrier = {
                                "engine": inst["engine"],
                                "ins": [],
                                "outs": [],
                                "name": f"{inst['name']}_xw{k}",
                                "opcode": "EventSemaphore",
                                "sync_info": {"on_update": [], "on_wait": [w]},
                            }
                            if "debug" in inst:
                                carrier["debug"] = inst["debug"]
                            out.append(carrier)
                        si["on_wait"] = [waits[-1]]
                out.append(inst)
            blk["instructions"] = out
    return orjson.dumps(bir) if changed else bir_json


def _compile_bir_kernel_split(bir_json, tmpdir, neff_name="file.neff"):
    return _orig_compile_bir_kernel(_split_multiwait_bir(bir_json), tmpdir, neff_name)


bass_utils.compile_bir_kernel = _compile_bir_kernel_split
bass2jax.compile_bir_kernel = _compile_bir_kernel_split

N, D, H, O, E = 16384, 1024, 256, 1024, 8
NCORES = 8
NTOK = N // NCORES  # tokens per core
P = 128
T = 128  # token block size
NB = NTOK // T  # token blocks per core
DC = D // P  # d chunks (contraction, layer 1)
EJ = E * H  # packed hidden width
NEJ = EJ // P  # ej chunks (contraction, layer 2)
JC_PER_E = H // P  # ej chunks per expert
OH = O // 2  # layer-2 output half width (one PSUM bank)

FP = mybir.dt.float32
AF = mybir.ActivationFunctionType


def _build_nc():
    nc = bass.Bass()
    xT = nc.dram_tensor("xT", [D, NTOK], FP, kind="ExternalInput")
    W1p = nc.dram_tensor("W1p", [D, EJ], FP, kind="ExternalInput")
    Wg = nc.dram_tensor("Wg", [D, E], FP, kind="ExternalInput")
    W2p = nc.dram_tensor("W2p", [EJ, O], FP, kind="ExternalInput")
    b1h = nc.dram_tensor("b1h", [P, NEJ], FP, kind="ExternalInput")
    bgh = nc.dram_tensor("bgh", [E, 1], FP, kind="ExternalInput")
    b2 = nc.dram_tensor("b2", [E, O], FP, kind="ExternalInput")
    out = nc.dram_tensor("out", [NTOK, O], FP, kind="ExternalOutput")

    with tile.TileContext(nc) as tc, ExitStack() as ctx:
        const = ctx.enter_context(tc.tile_pool(name="const", bufs=1))
        W1s = const.tile([P, DC, EJ], FP)
        nc.sync.dma_start(W1s[:], W1p.rearrange("(dc p) ej -> p dc ej", p=P))
        W2s = const.tile([P, NEJ, O], FP)
        nc.sync.dma_start(W2s[:], W2p.rearrange("(ec p) o -> p ec o", p=P))
        Wgs = const.tile([P, DC, E], FP)
        nc.sync.dma_start(Wgs[:], Wg.rearrange("(dc p) e -> p dc e", p=P))
        b1s = const.tile([P, NEJ], FP)
        nc.sync.dma_start(b1s[:], b1h[:])
        bgs = const.tile([E, 1], FP)
        nc.sync.dma_start(bgs[:], bgh[:])
        b2s = const.tile([E, O], FP)
        nc.sync.dma_start(b2s[:], b2[:])
        ones8 = const.tile([E, 1], FP)
        nc.gpsimd.memset(ones8[:], 1.0)

        dpool = ctx.enter_context(tc.tile_pool(name="dram", bufs=2, space="DRAM"))
        xpool = ctx.enter_context(tc.tile_pool(name="xts", bufs=2))
        gpool = ctx.enter_context(tc.tile_pool(name="gelu", bufs=3))
        shpool = ctx.enter_context(tc.tile_pool(name="sh", bufs=2))
        bcpool = ctx.enter_context(tc.tile_pool(name="bc", bufs=2))
        epool = ctx.enter_context(tc.tile_pool(name="expp", bufs=2))
        opool = ctx.enter_context(tc.tile_pool(name="outp", bufs=2))
        rpool = ctx.enter_context(tc.tile_pool(name="rcp", bufs=2))
        ps_h = ctx.enter_context(tc.tile_pool(name="ps_h", bufs=2, space="PSUM"))
        ps_g = ctx.enter_context(tc.tile_pool(name="ps_g", bufs=2, space="PSUM"))
        ps_s = ctx.enter_context(tc.tile_pool(name="ps_s", bufs=1, space="PSUM"))
        ps_o = ctx.enter_context(tc.tile_pool(name="ps_o", bufs=2, space="PSUM"))

        for blk in range(NB):
            t0 = blk * T
            xts = xpool.tile([P, DC, T], FP, name=f"xts{blk}", tag="xts")
            nc.sync.dma_start(
                xts[:], xT[:, t0 : t0 + T].rearrange("(dc p) t -> p dc t", p=P)
            )

            # gate logits (transposed): gt[e, t]
            gt = ps_g.tile([E, T], FP, name=f"gt{blk}", tag="gt")
            for dc in range(DC):
                nc.tensor.matmul(
                    gt[:],
                    Wgs[:, dc, :],
                    xts[:, dc, :],
                    start=(dc == 0),
                    stop=(dc == DC - 1),
                )
            expv = epool.tile([E, T], FP, name=f"exp{blk}", tag="exp")
            nc.scalar.activation(expv[:], gt[:], AF.Exp, bias=bgs[:, 0:1])
            # denominator, landed in token-partition layout via K=8 matmul
            s = ps_s.tile([T, 1], FP, name=f"s{blk}", tag="s")
            nc.tensor.matmul(s[:], expv[:], ones8[:], start=True, stop=True)
            rcp = rpool.tile([T, 1], FP, name=f"rcp{blk}", tag="rcp")
            nc.vector.reciprocal(rcp[:], s[:])
            # broadcast exp rows across partitions for the hidden scaling
            # (partition-stride-0 DMA only legal from DRAM -> bounce there)
            expd = dpool.tile([E, T], FP, name=f"expd{blk}", tag="expd")
            nc.sync.dma_start(expd[:], expv[:])
            bc = bcpool.tile([P, E, T], FP, name=f"bc{blk}", tag="bc")
            for e in range(E):
                nc.sync.dma_start(bc[:, e, :], expd[e : e + 1, :].to_broadcast((P, T)))

            # layer 1 + gelu + gate-scale, chunk by chunk
            sh = shpool.tile([P, NEJ, T], FP, name=f"sh{blk}", tag="sh")
            for ejc in range(NEJ):
                ht = ps_h.tile([P, T], FP, name=f"ht{blk}_{ejc}", tag="ht")
                for dc in range(DC):
                    nc.tensor.matmul(
                        ht[:],
                        W1s[:, dc, ejc * P : (ejc + 1) * P],
                        xts[:, dc, :],
                        start=(dc == 0),
                        stop=(dc == DC - 1),
                    )
                g = gpool.tile([P, T], FP, name=f"g{blk}_{ejc}", tag="g")
                nc.scalar.activation(g[:], ht[:], AF.Gelu, bias=b1s[:, ejc : ejc + 1])
                nc.vector.tensor_tensor(
                    sh[:, ejc, :], g[:], bc[:, ejc // JC_PER_E, :], mybir.AluOpType.mult
                )

            # layer 2: accumulate all ej chunks, seeded with expT @ b2
            outsb = opool.tile([P, O], FP, name=f"o{blk}", tag="o")
            for half in range(2):
                o0 = half * OH
                ops = ps_o.tile([P, OH], FP, name=f"ops{blk}_{half}", tag="ops")
                nc.tensor.matmul(
                    ops[:], expv[:], b2s[:, o0 : o0 + OH], start=True, stop=False
                )
                for ejc in range(NEJ):
                    nc.tensor.matmul(
                        ops[:],
                        sh[:, ejc, :],
                        W2s[:, ejc, o0 : o0 + OH],
                        start=False,
                        stop=(ejc == NEJ - 1),
                    )
                nc.scalar.activation(
                    outsb[:, o0 : o0 + OH], ops[:], AF.Copy, scale=rcp[:]
                )
            nc.sync.dma_start(out[t0 : t0 + T, :], outsb[:])
    return nc


_CACHE = {}


def kernel(**inputs):
    x = np.asarray(inputs["x"], dtype=np.float32)
    W1 = np.asarray(inputs["W1"], dtype=np.float32)
    b1 = np.asarray(inputs["b1"], dtype=np.float32)
    W2 = np.asarray(inputs["W2"], dtype=np.float32)
    b2 = np.asarray(inputs["b2"], dtype=np.float32)
    Wg = np.asarray(inputs["Wg"], dtype=np.float32)
    bg = np.asarray(inputs["bg"], dtype=np.float32)

    W1p = np.ascontiguousarray(W1.transpose(1, 0, 2).reshape(D, EJ))
    W2p = np.ascontiguousarray(W2.reshape(EJ, O))
    b1h = np.ascontiguousarray(b1.reshape(EJ).reshape(NEJ, P).T)
    bgh = np.ascontiguousarray(bg.reshape(E, 1))

    if "nc" not in _CACHE:
        _CACHE["nc"] = _build_nc()
    nc = _CACHE["nc"]

    in_maps = []
    for c in range(NCORES):
        xs = x[c * NTOK : (c + 1) * NTOK]
        in_maps.append(
            {
                "xT": np.ascontiguousarray(xs.T),
                "W1p": W1p,
                "Wg": Wg,
                "W2p": W2p,
                "b1h": b1h,
                "bgh": bgh,
                "b2": b2,
            }
        )

    res = run_bass_kernel_spmd(nc, in_maps, list(range(NCORES)))
    kernel.last = res
    return np.concatenate([res.results[c]["out"] for c in range(NCORES)], axis=0)
